# revision 87
# baseline (speedup 1.0000x reference)
"""GroupedQueryAttention Trainium2 kernel (8 NeuronCores, raw Bass).

Problem: B=4, S=1024, HID=2048, NH=32 q-heads, NKV=8 kv-heads, HD=64,
RoPE + causal softmax attention + out-projection.

Sharding: 8 cores = 4 batches x 2 head-groups. Each core handles one batch
and 16 q-heads / 4 kv-heads, computing a partial output (its head-group's
contribution through Wo); the host sums the two partials per batch.

v2: all matmul paths in bf16 (1 cycle/row on the PE array vs the 2-4x
penalty fp32 pays on K=64 / M=65 tiles), K-projection hoisted before
Q-projection, Q-projection matmuls interleaved into the attention unit
loop (stage B starts after ~2 of 16 projection tiles instead of all 16),
and the softmax reciprocal switched to reciprocal_approx_fast.

Per-core pipeline:
  A) V rows-projection into all 8 PSUM banks -> V_aug (ones column);
     K^T projection + RoPE -> replicated KT; Q^T f=0 tiles + RoPE.
  B) per (head, 512-row q-block): scores S^T = K^T.T Q^T -> exp(bf16) ->
     causal mask multiply (diagonal tiles) -> O^T_aug = V_aug^T expS^T;
     row 64 = denominator; bcast via K=1 matmul; approx-reciprocal;
     normalize into OT. Remaining Q^T projection tiles stream through the
     same PE queue in 8-matmul chunks.
  C) out = OT.T @ Wo per (col-slice, row-tile), staged fp32, DMA'd out.

Explicit per-engine scheduling (one semaphore per producer engine,
python-side counter bookkeeping).
"""

import numpy as np
import concourse.bass as bass
import concourse.mybir as mybir
from concourse.bass_utils import run_bass_kernel_spmd

F32 = mybir.dt.float32
F32R = mybir.dt.float32r
BF16 = mybir.dt.bfloat16
AF = mybir.ActivationFunctionType

B, S, HID = 4, 1024, 2048
NH, NKV, HD = 32, 8, 64
NHC, NKVC = NH // 2, NKV // 2      # per-core: 16 q heads, 4 kv heads
KT = HID // 128                     # 16 k-tiles over hidden dim
THETA = 10000.0

_CACHE = {}


def _build_nc():
    nc = bass.Bass(dynamic_dma_scratch_size=2048)

    # ---- DRAM params (per-core views, host pre-sharded/pre-transposed) ----
    ht_d = nc.declare_dram_parameter("ht", [HID, S], BF16, isOutput=False)
    wq_d = nc.declare_dram_parameter("wq", [8, HID, 128], BF16, isOutput=False)
    wk_d = nc.declare_dram_parameter("wk", [2, HID, 128], BF16, isOutput=False)
    wv_d = nc.declare_dram_parameter("wv", [HID, 256], BF16, isOutput=False)
    wo_d = nc.declare_dram_parameter("wo", [NHC * HD, HID], BF16, isOutput=False)
    cosd_d = nc.declare_dram_parameter("cosd", [128, S], F32, isOutput=False)
    sinr_d = nc.declare_dram_parameter("sinr", [128, S], F32, isOutput=False)
    mask_d = nc.declare_dram_parameter("masks", [128, 4 * 512], BF16, isOutput=False)
    ident_d = nc.declare_dram_parameter("ident", [128, 128], BF16, isOutput=False)
    out_d = nc.declare_dram_parameter("out", [S, HID], F32, isOutput=True)

    # ---- SBUF map (bytes per partition; SWDGE scratch pinned at [0, 2048)) ----
    off = [2048]

    sb_offs = {}

    def sb(name, shape, dt, align=32, alias_r=False):
        o = (off[0] + align - 1) // align * align
        h = nc.alloc_sbuf_tensor_at(name, shape, dt, offset=o)
        hr = (nc.alloc_sbuf_tensor_at(name + "_r", shape, F32R, offset=o)
              if alias_r else None)
        esz = 2 if dt in (BF16, mybir.dt.float16, mybir.dt.int16) else 4
        n = 1
        for s in shape[1:]:
            n *= s
        sb_offs[name] = o
        off[0] = o + n * esz
        return (h, hr) if alias_r else h

    QT = sb("QT", [128, 8, 1024], BF16)            # 16K
    # K^T zero-padded per head half: KT0 = [K | 0], KT1 = [0 | K] so scores
    # matmuls contract over the full 128 partitions (K=64 shapes put the PE
    # array into half-row-group mode, slowing the whole phase 2x)
    KT0 = sb("KT0", [128, 4, 1024], BF16)          # 8K
    KT1 = sb("KT1", [128, 4, 1024], BF16)          # 8K
    VA = sb("Vaug", [128, 8, 4, 128], BF16)        # 8K (ones col 64, 0 pad)
    cosd = sb("cosd", [128, 1024], F32)            # 4K
    sinr = sb("sinr", [128, 1024], F32)            # 4K
    onesf = sb("ones", [128, 128], BF16)           # row-64 selector (bf16)
    zb = sb("zb", [128, 1], F32)
    # softmax denominator row buffers (partition 64, 3 slots for lag-3)
    denf = sb("den", [128, 3, 512], BF16)          # 1/den (NR out, bf16)
    denb = sb("denb", [128, 3, 512], BF16)         # den as bf16
    denbi = nc.alloc_sbuf_tensor_at("denb_i16", [128, 3, 512], mybir.dt.int16,
                                    offset=sb_offs["denb"])
    r0b = sb("r0b", [128, 3, 512], BF16)           # recip seed
    r0i = nc.alloc_sbuf_tensor_at("r0b_i16", [128, 3, 512], mybir.dt.int16,
                                  offset=sb_offs["r0b"])
    ttf = sb("ttf", [128, 3, 512], F32)            # NR scratch t
    ttb = sb("ttb", [128, 3, 512], BF16)           # NR scratch u
    recf = sb("recip", [128, 2, 512], F32)         # 4K (rows 0-63)
    OT = sb("OT", [128, 8, 1024], BF16)            # 16K
    rsc = sb("rsc", [128, 512], F32)               # 2K rope scratch (rot*sin)
    rsc2 = sb("rsc2", [128, 512], F32)             # 2K rope scratch (x*cos)
    HT = sb("HT", [128, 16, 1024], BF16)           # 32K
    wqs = sb("wq_s", [128, 2, 16, 128], BF16)      # 8K
    wks = sb("wk_s", [128, 2, 16, 128], BF16)      # 8K
    wvs = sb("wv_s", [128, 16, 256], BF16)         # 8K
    ktmp = sb("ktmp", [128, 2, 1024], BF16)        # 4K
    wos = sb("wo_s", [128, 2, 8, 512], BF16)       # 8K
    stg = sb("stg", [128, 4, 512], F32)            # 8K
    masks = sb("masks_s", [128, 4, 512], BF16)     # 4K additive 0/-50
    ident = sb("ident_s", [128, 128], BF16)        # 256B identity
    exS = sb("expS", [128, 6, 512], BF16)          # 6K
    exSi = nc.alloc_sbuf_tensor_at("expS_i16", [128, 6, 512], mybir.dt.int16,
                                   offset=sb_offs["expS"])
    assert off[0] <= 208 * 1024, off[0]

    # ---- PSUM: 8 banks ----
    P = [nc.alloc_psum_tensor(f"pp{i}", [128, 512], F32) for i in range(2)]
    SBk = [nc.alloc_psum_tensor(f"ps{i}", [128, 512], F32) for i in range(2)]
    OB = [nc.alloc_psum_tensor(f"po{i}", [128, 512], F32) for i in range(2)]
    BBk = [nc.alloc_psum_tensor(f"pb{i}", [128, 512], F32) for i in range(2)]
    banks8 = [P[0], P[1], SBk[0], SBk[1], OB[0], OB[1], BBk[0], BBk[1]]
    SBp = [SBk[0], SBk[1], BBk[0]]          # stage-B scores pool, tile i%3
    OBp = [OB[0], OB[1], P[0], BBk[1]]      # stage-B output pool, unit u%4
    BCB = P[1]                              # stage-B bcast bank

    # ---- per-engine op lists + counters ----
    prog = {e: [] for e in ("pe", "act", "dve", "gp")}
    waited = {e: {} for e in ("pe", "act", "dve", "gp", "sp")}
    ctr = {"load": 0, "pe": 0, "act": 0, "dve": 0, "gp": 0, "store": 0}
    bank_rel = {}  # id(psum handle) -> (sem_name, count)

    def wait(e, sem_name, val):
        if val is None or val <= 0:
            return
        if waited[e].get(sem_name, 0) >= val:
            return
        waited[e][sem_name] = val
        prog[e].append(("w", sem_name, val))

    def wait_bank(e, bank):
        r = bank_rel.get(id(bank))
        if r:
            wait(e, r[0], r[1])

    def op(e, fn, inc=None):
        prog[e].append(("o", fn, inc))
        if inc:
            ctr[inc[0]] += inc[1]
            return ctr[inc[0]]
        return None

    def dop(fn, inc=False):
        return op("dve", fn, ("dve", 1) if inc else None)

    def mm(bank_ap, lhsT, rhs, start, stop):
        def fn(bank_ap=bank_ap, lhsT=lhsT, rhs=rhs, start=start, stop=stop):
            return nc.tensor.matmul(bank_ap, lhsT, rhs, start=start, stop=stop,
                                    skip_group_check=True)
        return fn

    # ================= SP: input loads (HWDGE FIFO, in order) =================
    loads = []           # (dst_ap, src_ap)
    gates = {}           # load index -> (sem, count)

    def load(dst, src):
        loads.append((dst, src))
        ctr["load"] += 1
        return ctr["load"]

    n_ht = [load(HT[:, 0:4, :], ht_d[0:512, :].rearrange("(o p) r -> p o r", p=128))]
    n_wv = load(wvs[:], wv_d[:].rearrange("(o p) v -> p o v", p=128))
    n_wk = [load(wks[:, i], wk_d[i].rearrange("(o p) f -> p o f", p=128))
            for i in range(2)]
    n_ht += [load(HT[:, 4 * g:4 * g + 4, :],
                  ht_d[512 * g:512 * (g + 1), :].rearrange("(o p) r -> p o r", p=128))
             for g in range(1, 4)]
    n_cos = load(cosd[:], cosd_d[:])
    n_sin = load(sinr[:], sinr_d[:])
    n_wq = {}
    n_wq[0] = load(wqs[:, 0], wq_d[0].rearrange("(o p) f -> p o f", p=128))
    n_wq[1] = load(wqs[:, 1], wq_d[1].rearrange("(o p) f -> p o f", p=128))
    n_masks = load(masks[:], mask_d[:].rearrange("p (a b) -> p a b", a=4))
    n_ident = load(ident[:], ident_d[:])
    wq_gate_slots = {}
    for qf in range(2, 8):
        wq_gate_slots[qf] = len(loads)
        n_wq[qf] = load(wqs[:, qf % 2], wq_d[qf].rearrange("(o p) f -> p o f", p=128))
    n_wo = {}
    n_wo[0] = load(wos[:, 0], wo_d[:, 0:512].rearrange("(o p) c -> p o c", p=128))
    n_wo[1] = load(wos[:, 1], wo_d[:, 512:1024].rearrange("(o p) c -> p o c", p=128))
    wo_gate_slots = {}
    for cs in (2, 3):
        wo_gate_slots[cs] = len(loads)
        n_wo[cs] = load(wos[:, cs % 2],
                        wo_d[:, 512 * cs:512 * (cs + 1)].rearrange("(o p) c -> p o c", p=128))

    # ================= DVE init =================
    dop(lambda: nc.vector.memset(zb[:], 0.0))
    # "ones" is a row-64 selector: bcast out[m,n] = den[64,n] with K=M=128
    dop(lambda: nc.vector.memset(onesf[:], 0.0))
    dop(lambda: nc.vector.memset(onesf[64:65, :], 1.0))
    dop(lambda: nc.vector.memset(denf[:], 0.0))
    dop(lambda: nc.vector.memset(KT0[64:128, :, :], 0.0))
    dop(lambda: nc.vector.memset(KT1[0:64, :, :], 0.0))
    dop(lambda: nc.vector.memset(VA[:, :, :, 65:128], 0.0))
    base_done = dop(lambda: nc.vector.memset(VA[:, :, :, 64:65], 1.0), True)

    # ================= stage A: V projection (all 8 banks) =================
    v_stop = {}
    for k in range(KT):
        if k == 0:
            wait("pe", "load", 16 * max(n_ht[0], n_wv))
        else:
            wait("pe", "load", 16 * n_ht[k // 4])
        for rt in range(8):
            inc = ("pe", 1) if k == KT - 1 else None
            n = op("pe", mm(banks8[rt][:, 0:256], HT[:, k, 128 * rt:128 * rt + 128],
                            wvs[:, k, :], k == 0, k == KT - 1), inc)
            if k == KT - 1:
                v_stop[rt] = n

    # DVE: V_aug copies (PSUM f32 -> bf16 rows layout)
    vaug_done = {}
    for rt in range(8):
        wait("dve", "pe", v_stop[rt])
        n = None
        for kvi in range(4):
            n = dop(lambda rt=rt, kvi=kvi: nc.vector.tensor_copy(
                out=VA[:, rt, kvi, 0:64], in_=banks8[rt][:, 64 * kvi:64 * kvi + 64]),
                kvi == 3)
        vaug_done[rt] = n
        bank_rel[id(banks8[rt])] = ("dve", n)

    # ================= rope helper (DVE) =================
    rope_srcs = [32, 0, 96, 64]

    # ================= stage A: K projection + rope =================
    # K tiles tk=0..3 (kf=tk//2, r=tk%2) on P[tk%2]
    kt_stop = {}
    kt_rope_done = {}
    ktrep_done = {}

    def emit_rope(bank, out_ap, rwin):
        """out_ap(bf16) = bank*cos + rotate_half(bank)*sin, f32 scratches."""
        for q in range(4):
            s0 = rope_srcs[q]
            dop(lambda q=q, s0=s0, bank=bank, rwin=rwin: nc.vector.tensor_mul(
                out=rsc[32 * q:32 * q + 32, :],
                in0=bank[s0:s0 + 32, :],
                in1=sinr[32 * q:32 * q + 32, rwin:rwin + 512]))
        dop(lambda bank=bank, rwin=rwin: nc.vector.tensor_mul(
            out=rsc2[:], in0=bank[:], in1=cosd[:, rwin:rwin + 512]))
        return dop(lambda out_ap=out_ap: nc.vector.tensor_add(
            out=out_ap, in0=rsc[:], in1=rsc2[:]), True)

    # ============ stage A: K projections (P0/P1 ping-pong) =========
    def emit_k_tile(tk):
        kf, r = tk // 2, tk % 2
        wait("pe", "load", 16 * n_wk[kf])
        if tk < 2:
            wait("pe", "dve", vaug_done[tk])
        else:
            wait("pe", "dve", kt_rope_done[tk - 2])
        n = None
        for k in range(KT):
            inc = ("pe", 1) if k == KT - 1 else None
            n = op("pe", mm(P[tk % 2][:], wks[:, kf, k, :],
                            HT[:, k, 512 * r:512 * r + 512], k == 0, k == KT - 1), inc)
        kt_stop[tk] = n

    def emit_k_rope(tk):
        kf, r = tk // 2, tk % 2
        wait("dve", "pe", kt_stop[tk])
        kt_rope_done[tk] = emit_rope(P[tk % 2], ktmp[:, kf, 512 * r:512 * r + 512],
                                     512 * r)
        if tk % 2 == 1:
            # both halves of ktmp[:, kf, :] ready -> fill the padded KT pair
            for hs in range(2):
                kv = 2 * kf + hs
                dop(lambda kv=kv, kf=kf, hs=hs: nc.vector.tensor_copy(
                    out=KT0[0:64, kv, :], in_=ktmp[64 * hs:64 * hs + 64, kf, :]))
                ktrep_done[kv] = dop(lambda kv=kv, kf=kf, hs=hs: nc.vector.tensor_copy(
                    out=KT1[64:128, kv, :], in_=ktmp[64 * hs:64 * hs + 64, kf, :]), True)

    wait("dve", "load", 16 * n_sin)
    emit_k_tile(0)
    emit_k_tile(1)
    emit_k_rope(0)
    emit_k_rope(1)
    emit_k_tile(2)
    emit_k_tile(3)
    emit_k_rope(2)
    emit_k_rope(3)

    # ============ stage A: Q projections (16 tiles, P0/P1 ping-pong) =======
    qt_stop = {}
    qt_rope_done = {}

    for t in range(16):
        f, r = t // 2, t % 2
        wait("pe", "load", 16 * n_wq[f])
        if t < 2:
            wait("pe", "dve", kt_rope_done[2 + t])
        else:
            wait("pe", "dve", qt_rope_done[t - 2])
        n = None
        for k in range(KT):
            inc = ("pe", 1) if k == KT - 1 else None
            n = op("pe", mm(P[t % 2][:], wqs[:, f % 2, k, :],
                            HT[:, k, 512 * r:512 * r + 512], k == 0, k == KT - 1), inc)
        qt_stop[t] = n
        wait("dve", "pe", n)
        qt_rope_done[t] = emit_rope(P[t % 2], QT[:, f, 512 * r:512 * r + 512],
                                    512 * r)
    bank_rel[id(P[0])] = ("dve", qt_rope_done[14])
    bank_rel[id(P[1])] = ("dve", qt_rope_done[15])

    # ================= stage B =================
    # Flat software-pipelined tile stream: scores run LAG tiles ahead of PVs
    # so the PE never waits on a fresh convert (keeping it continuously busy
    # holds the 2.4GHz p-state; idle gaps drop it to 1.2GHz for ~whole phase).
    units = [(h, Q) for h in range(NHC) for Q in range(2)]
    scores_n, pv_stop, bcast_n = {}, {}, {}
    exp_n, den_n, norm_n, recip_n, recf_n = {}, {}, {}, {}, {}
    conv_eng = {}
    LAG = 5

    def gop(fn, inc=False):
        return op("gp", fn, ("gp", 1) if inc else None)

    def emit_bcast(uu):
        # selector matmul broadcasts the 1/den row across all partitions
        wait("pe", "gp", recip_n[uu])
        wait_bank("pe", BCB)
        bcast_n[uu] = op("pe", mm(BCB[:], onesf[:],
                                  denf[:, uu % 3, :], True, True), ("pe", 1))

    def emit_normalize(uu):
        h, Q = units[uu]
        m, f = h % 2, h // 2
        # DVE: stage the broadcast reciprocal into SBUF, then OT = OB * recf
        wait("dve", "pe", bcast_n[uu])
        cn = dop(lambda uu=uu: nc.vector.tensor_copy(
            out=recf[0:64, uu % 2, :], in_=BCB[0:64, :]), True)
        n = dop(lambda uu=uu, m=m, f=f, Q=Q: nc.vector.tensor_mul(
            out=OT[64 * m:64 * m + 64, f, 512 * Q:512 * Q + 512],
            in0=OBp[uu % 4][0:64, :], in1=recf[0:64, uu % 2, :]), True)
        norm_n[uu] = n
        recf_n[uu] = cn
        bank_rel[id(OBp[uu % 4])] = ("dve", n)
        bank_rel[id(BCB)] = ("dve", cn)

    # Schraudolph exp in bf16 bit space: bf16_bits(exp(x)) ~ round(A16*x+B16).
    # One scale+bias+int16-convert op per tile -- no Exp ACTIVATE needed.
    # Diagonal (trimmed) tiles convert on DVE, full tiles on ACT.
    A16 = 128.0 / float(np.log(2.0))
    B16 = 16256.0 - 7.42

    def trim(c, Q):
        # causal trim: diagonal tile c covers only q >= 128*(c-4Q)
        return 128 * (c - 4 * Q) if c >= 4 * Q else 0

    tiles = []
    for u, (h, Q) in enumerate(units):
        for c in range(4 * Q + 4):
            tiles.append((u, c, Q, h))

    def emit_scores(i):
        u, c, Q, h = tiles[i]
        kv, m, f = h // 4, h % 2, h // 2
        lo = trim(c, Q)
        diag = c >= 4 * Q
        bank = SBp[i % 3]
        KTm = KT0 if m == 0 else KT1
        wait("pe", "dve", qt_rope_done[2 * f + Q])
        wait("pe", "dve", ktrep_done[kv])
        wait_bank("pe", bank)
        op("pe", mm(
            bank[:, lo:512],
            KTm[:, kv, 128 * c:128 * c + 128],
            QT[:, f, 512 * Q + lo:512 * Q + 512],
            True, not diag), ("pe", 1) if not diag else None)
        if diag:
            wait("pe", "load", 16 * n_ident)
            op("pe", mm(bank[:, lo:512], ident[:],
                        masks[:, c - 4 * Q, lo:512], False, True), ("pe", 1))
        scores_n[i] = ctr["pe"]
        # convert: diagonal (trimmed) tiles on DVE, full tiles on ACT
        eng = "dve" if diag else "act"
        conv_eng[i] = eng
        wait(eng, "pe", scores_n[i])
        if eng == "dve":
            exp_n[i] = dop(lambda i=i, lo=lo, bank=bank:
                           nc.vector.tensor_scalar(
                out=exSi[:, i % 6, lo:512], in0=bank[:, lo:512],
                scalar1=A16, scalar2=B16,
                op0=mybir.AluOpType.mult, op1=mybir.AluOpType.add), True)
        else:
            exp_n[i] = op("act", (lambda i=i, lo=lo, bank=bank:
                                  nc.scalar.activation(
                out=exSi[:, i % 6, lo:512], in_=bank[:, lo:512],
                func=AF.Copy, bias=B16, scale=A16)), ("act", 1))
        bank_rel[id(bank)] = (eng, exp_n[i])

    def emit_pv(i):
        u, c, Q, h = tiles[i]
        kv = h // 4
        lo = trim(c, Q)
        last = (c == 4 * Q + 3)
        if c == 0:
            # first PV of unit u: release path for this unit's OB bank
            if u >= 3:
                emit_bcast(u - 3)
                emit_normalize(u - 3)
            wait("pe", "dve", base_done)
            wait_bank("pe", OBp[u % 4])
        wait("pe", conv_eng[i], exp_n[i])
        wait("pe", "dve", vaug_done[c])
        inc = ("pe", 1) if last else None
        n = op("pe", mm(OBp[u % 4][:, lo:512], VA[:, c, kv, :],
                        exS[:, i % 6, lo:512], c == 0, last), inc)
        if not last:
            return
        pv_stop[u] = n
        # ACT: den row -> bf16 (bit seed source)
        if u >= 3:
            wait("act", "pe", bcast_n[u - 3])
        wait("act", "pe", pv_stop[u])
        den_n[u] = op("act", (lambda u=u: nc.scalar.copy(
            out=denb[64:65, u % 3, :], in_=OBp[u % 4][64:65, :])), ("act", 1))
        # 1/den on GPSIMD: bf16 bit-negate seed + one Newton step against the
        # bf16 den (rounding adds <0.2%, well within budget)
        wait("gp", "act", den_n[u])
        gop(lambda u=u: nc.gpsimd.tensor_scalar(
            out=r0i[64:65, u % 3, :], in0=denbi[64:65, u % 3, :],
            scalar1=-1.0, scalar2=32499.0,
            op0=mybir.AluOpType.mult, op1=mybir.AluOpType.add))
        gop(lambda u=u: nc.gpsimd.tensor_mul(
            out=ttf[64:65, u % 3, :], in0=denb[64:65, u % 3, :],
            in1=r0b[64:65, u % 3, :]))
        gop(lambda u=u: nc.gpsimd.tensor_scalar(
            out=ttb[64:65, u % 3, :], in0=ttf[64:65, u % 3, :],
            scalar1=-1.0, scalar2=2.0,
            op0=mybir.AluOpType.mult, op1=mybir.AluOpType.add))
        gop(lambda u=u: nc.gpsimd.tensor_mul(
            out=denf[64:65, u % 3, :], in0=ttb[64:65, u % 3, :],
            in1=r0b[64:65, u % 3, :]))
        # tiny trailing op carries the semaphore so the inc can't race the
        # 1/den write completing
        recip_n[u] = gop(lambda u=u: nc.gpsimd.tensor_copy(
            out=ttf[64:65, u % 3, 0:8], in_=ttf[64:65, u % 3, 0:8]), True)

    for i in range(len(tiles)):
        emit_scores(i)
        if i >= LAG:
            emit_pv(i - LAG)
    for i in range(len(tiles) - LAG, len(tiles)):
        emit_pv(i)

    # ================= load gates =================
    for qf, slot in wq_gate_slots.items():
        gates[slot] = ("pe", qt_stop[2 * (qf - 2) + 1])

    # ================= stage C (on the SB score banks; per-fi norm waits;
    # the first two tiles' fi 0..6 interleave with the last three
    # normalizations so the PE never idles on the tail GP chains) ==========
    cgrp = {}
    sidx = 0

    def c_tile_mms(cs, rt, fi_lo, fi_hi):
        for fi in range(fi_lo, fi_hi):
            wait("pe", "dve", norm_n[4 * fi + 3])
            inc = ("pe", 1) if fi == 7 else None
            n = op("pe", mm(SBk[rt % 2][:], OT[:, fi, 128 * rt:128 * rt + 128],
                            wos[:, cs % 2, fi, :], fi == 0, fi == 7), inc)
            if fi == 7:
                cgrp[(cs, rt)] = n

    def c_tile_drain(cs, rt):
        slot = sidx % 4
        wait("dve", "pe", cgrp[(cs, rt)])
        if sidx >= 4:
            wait("dve", "store", 16 * (sidx - 3))
        cn = dop(lambda rt=rt, slot=slot: nc.vector.tensor_copy(
            out=stg[:, slot, :], in_=SBk[rt % 2][:]), True)
        bank_rel[id(SBk[rt % 2])] = ("dve", cn)
        wait("act", "dve", cn)
        op("act", (lambda cs=cs, rt=rt, slot=slot: nc.scalar.dma_start(
            out=out_d[128 * rt:128 * rt + 128, 512 * cs:512 * cs + 512],
            in_=stg[:, slot, :])), ("store", 16))

    emit_bcast(len(units) - 3)
    emit_normalize(len(units) - 3)
    emit_bcast(len(units) - 2)
    emit_normalize(len(units) - 2)
    emit_bcast(len(units) - 1)
    emit_normalize(len(units) - 1)
    for cs in range(4):
        wait("pe", "load", 16 * n_wo[cs])
        for rt in range(8):
            wait_bank("pe", SBk[rt % 2])
            c_tile_mms(cs, rt, 0, 8)
            c_tile_drain(cs, rt)
            sidx += 1
    wait("act", "store", 16 * 32)
    for cs, slot in wo_gate_slots.items():
        gates[slot] = ("pe", cgrp[(cs - 2, 7)])

    # ================= emit =================
    sems = {}
    with (
        nc.Block() as block,
        nc.semaphore("s_load") as s_load,
        nc.semaphore("s_pe") as s_pe,
        nc.semaphore("s_act") as s_act,
        nc.semaphore("s_dve") as s_dve,
        nc.semaphore("s_gp") as s_gp,
        nc.semaphore("s_store") as s_store,
    ):
        sems.update({"load": s_load, "pe": s_pe, "act": s_act,
                     "dve": s_dve, "gp": s_gp, "store": s_store})

        @block.sync
        def _(sync):
            for i, (dst, src) in enumerate(loads):
                g = gates.get(i)
                if g:
                    sync.wait_ge(sems[g[0]], g[1])
                sync.dma_start(out=dst, in_=src).then_inc(s_load, 16)

        def run(eng, lst):
            for item in lst:
                if item[0] == "w":
                    eng.wait_ge(sems[item[1]], item[2])
                else:
                    inst = item[1]()
                    if item[2] is not None:
                        sem, ninc = item[2]
                        inst.then_inc(sems[sem], ninc)

        @block.tensor
        def _(pe):
            run(pe, prog["pe"])

        @block.scalar
        def _(act):
            run(act, prog["act"])

        @block.vector
        def _(dve):
            run(dve, prog["dve"])

        @block.gpsimd
        def _(gp):
            run(gp, prog["gp"])

    return nc


def _host_prep(hidden_states, position_ids, Wq, Wk, Wv, Wo):
    """Build the 8 per-core input maps (matmul operands in bf16)."""
    import ml_dtypes
    bf16 = ml_dtypes.bfloat16

    pos = position_ids.astype(np.float32)
    inv = 1.0 / (THETA ** (np.arange(0, HD, 2, dtype=np.float32) / HD))
    ang = pos[:, None] * inv[None, :]                  # [S, 32]
    emb = np.concatenate([ang, ang], axis=1)           # [S, 64]
    cos_t = np.ascontiguousarray(np.cos(emb).T.astype(np.float32))   # [64, S]
    sin_t = np.sin(emb).T.astype(np.float32)
    cosd = np.ascontiguousarray(np.concatenate([cos_t, cos_t], axis=0))
    sgn = np.where(np.arange(HD) < HD // 2, -1.0, 1.0).astype(np.float32)
    sin_s = sin_t * sgn[:, None]
    sinr = np.ascontiguousarray(np.concatenate([sin_s, sin_s], axis=0))

    kc = np.arange(128)[:, None]
    qr = np.arange(512)[None, :]
    masks = np.ascontiguousarray(np.concatenate(
        [np.where(qr >= 128 * d + kc, 0.0, -50.0) for d in range(4)],
        axis=1).astype(bf16))
    ident = np.ascontiguousarray(np.eye(128, dtype=np.float32).astype(bf16))

    scale = np.float32(HD ** -0.5)
    in_maps = []
    for cid in range(8):
        b, hg = cid // 2, cid % 2
        ht = np.ascontiguousarray(hidden_states[b].T.astype(bf16))
        wq = np.ascontiguousarray(
            (Wq[:, hg * 1024:(hg + 1) * 1024] * scale)
            .reshape(HID, 8, 128).transpose(1, 0, 2).astype(bf16))
        wk = np.ascontiguousarray(
            Wk[:, hg * 256:(hg + 1) * 256].reshape(HID, 2, 128)
            .transpose(1, 0, 2).astype(bf16))
        wv = np.ascontiguousarray(Wv[:, hg * 256:(hg + 1) * 256].astype(bf16))
        wo = np.ascontiguousarray(Wo[hg * 1024:(hg + 1) * 1024, :].astype(bf16))
        in_maps.append({"ht": ht, "wq": wq, "wk": wk, "wv": wv, "wo": wo,
                        "cosd": cosd, "sinr": sinr, "masks": masks,
                        "ident": ident})
    return in_maps


def kernel(hidden_states, attention_mask, position_ids, Wq, Wk, Wv, Wo,
           _trace=False, _trace_kwargs=None):
    if "nc" not in _CACHE:
        _CACHE["nc"] = _build_nc()
    nc = _CACHE["nc"]
    in_maps = _host_prep(np.asarray(hidden_states), np.asarray(position_ids),
                         np.asarray(Wq), np.asarray(Wk), np.asarray(Wv),
                         np.asarray(Wo))
    kw = {}
    if _trace:
        kw = {"trace": True}
        if _trace_kwargs:
            kw.update(_trace_kwargs)
    res = run_bass_kernel_spmd(nc, in_maps, list(range(8)), **kw)
    outs = [res.results[cid]["out"] for cid in range(8)]
    full = np.empty((B, S, HID), dtype=np.float32)
    for b in range(B):
        full[b] = outs[2 * b] + outs[2 * b + 1]
    if _trace:
        kernel._last_result = res
    return full
